# revision 1
# baseline (speedup 1.0000x reference)
"""Multi-head attention (B=2, S=2048, H=1024, 16 heads x 64) on 8 TRN2 cores.

Sharding: data-parallel over batch (cores 0-3 -> b=0, cores 4-7 -> b=1),
tensor-parallel over heads (4 heads / core).  Each core computes its
heads' full attention; the host assembles the output.

ACT-saturating schedule: the exp over 16.8M scores/core is the hard
floor (~121us on the scalar engine at 1 elem/lane/cycle @1.2GHz + per-
call overhead).  Scores land in two ping-pong PSUM regions (scA
[128,4,512] -> N=2048 exp calls, scB [128,2,512] -> N=1024) so the
scalar engine never idles; projections, stage B (V has a ones column
yielding the softmax denominator for free), PE transposes and the
divide epilogue are queued and drained between score fills.  Mask is
applied multiplicatively after exp on DVE (fp16 2x mode), all compute
in fp16 with fp32 accumulation.

Measured (test.py reps-slope): 120,988 ns/body, rel err 3.9e-4
(baseline fused kernel: 205,370 ns).
"""

import sys

if "/opt/trn_rl_repo" not in sys.path:
    sys.path.insert(0, "/opt/trn_rl_repo")

from collections import deque
from contextlib import ExitStack

import numpy as np

import concourse.bass as bass
import concourse.tile as tile
from concourse import bacc, mybir
from concourse.bass_utils import run_bass_kernel_spmd
from concourse.masks import make_identity

F32 = mybir.dt.float32
FP16 = mybir.dt.float16

B, S, H = 2, 2048, 1024
NH, HD = 16, 64
NCORES = 8
CORES_PER_B = NCORES // B
LHEADS = NH // CORES_PER_B
COLS = LHEADS * HD
HC = H // 128
SBLK = 512
NSB = S // SBLK
JT = S // 128

GROUPS = [(0, 1), (2,), (3, 4), (5,), (6, 7), (8,), (9, 10), (11,),
          (12, 13), (14,), (15,)]


def build_program(reps=1, fp16=True, pbig_bufs=11, psml_bufs=13, gmask=3,
                  **_kw):
    WDT = FP16
    PDT = FP16
    nc = bacc.Bacc("TRN2", target_bir_lowering=False, debug=False)

    xT = nc.dram_tensor("xT", [H, S], WDT, kind="ExternalInput").ap()
    maskp = nc.dram_tensor("maskp", [S, S], PDT, kind="ExternalInput").ap()
    wq = nc.dram_tensor("wq", [H, COLS], WDT, kind="ExternalInput").ap()
    wk = nc.dram_tensor("wk", [H, COLS], WDT, kind="ExternalInput").ap()
    wv = nc.dram_tensor("wv", [H, COLS], WDT, kind="ExternalInput").ap()
    bq = nc.dram_tensor("bq", [COLS, 1], F32, kind="ExternalInput").ap()
    bk = nc.dram_tensor("bk", [COLS, 1], F32, kind="ExternalInput").ap()
    bv = nc.dram_tensor("bv", [1, COLS], WDT, kind="ExternalInput").ap()
    ones_d = nc.dram_tensor("ones_d", [1, 128], WDT, kind="ExternalInput").ap()
    out = nc.dram_tensor("out", [S, COLS], F32, kind="ExternalOutput").ap()

    with tile.TileContext(nc) as tc:
      for _rep in range(reps):
        with ExitStack() as ctx:
            persist = ctx.enter_context(tc.tile_pool(name="persist", bufs=1))
            wpool = ctx.enter_context(tc.tile_pool(name="wpool", bufs=1))
            xpool = ctx.enter_context(tc.tile_pool(name="xpool", bufs=1))
            mpool = ctx.enter_context(tc.tile_pool(name="mpool", bufs=2))
            ppool = ctx.enter_context(tc.tile_pool(name="ppool", bufs=1))
            opool = ctx.enter_context(tc.tile_pool(name="opool", bufs=2))
            upool = ctx.enter_context(tc.tile_pool(name="upool", bufs=2))
            rpool = ctx.enter_context(tc.tile_pool(name="rpool", bufs=2))
            psum = ctx.enter_context(tc.tile_pool(name="psum", bufs=1, space="PSUM"))

            ident = persist.tile([128, 128], F32)
            make_identity(nc, ident[:])
            ones1 = persist.tile([1, 128], WDT)
            bq_sb = persist.tile([128, 2], F32)
            bk_sb = persist.tile([128, 2], F32)
            bv_sb = persist.tile([1, COLS], WDT)

            QT = [persist.tile([128, S], WDT, name=f"QT{p}") for p in range(2)]
            KT = [persist.tile([128, S], WDT, name=f"KT{p}") for p in range(2)]
            Vp = persist.tile([128, JT, LHEADS, 66], PDT)
            nc.gpsimd.memset(Vp[:, :, :, 64:65], 1.0)

            wk_sb = wpool.tile([128, HC, COLS], WDT)
            wq_sb = wpool.tile([128, HC, COLS], WDT)
            wv_sb = wpool.tile([128, HC, COLS], WDT)
            xts = [xpool.tile([128, HC, SBLK], WDT, name=f"xt{sb}")
                   for sb in range(NSB)]

            def load_consts():
                nc.sync.dma_start(ones1[:], ones_d[:])
                for hp in range(2):
                    nc.sync.dma_start(
                        bq_sb[:, hp : hp + 1], bq[hp * 128 : hp * 128 + 128, :]
                    )
                    nc.sync.dma_start(
                        bk_sb[:, hp : hp + 1], bk[hp * 128 : hp * 128 + 128, :]
                    )
                nc.sync.dma_start(bv_sb[:], bv[:])

            def load_w(w_sb, w_dram):
                for hc in range(HC):
                    nc.sync.dma_start(
                        w_sb[:, hc, :], w_dram[hc * 128 : (hc + 1) * 128, :]
                    )

            def load_xt(sb):
                for hc in range(HC):
                    nc.sync.dma_start(
                        xts[sb][:, hc, :],
                        xT[hc * 128 : (hc + 1) * 128, sb * SBLK : (sb + 1) * SBLK],
                    )

            def proj_kq(which, sb, hp):
                w_sb, b_sb, dst = (
                    (wk_sb, bk_sb, KT) if which == "k" else (wq_sb, bq_sb, QT)
                )
                sl = slice(sb * SBLK, (sb + 1) * SBLK)
                cs = slice(hp * 128, hp * 128 + 128)
                pk = psum.tile([128, SBLK], F32, name="pk", tag="bp")
                for hc in range(HC):
                    nc.tensor.matmul(
                        pk[:], w_sb[:, hc, cs], xts[sb][:, hc, :],
                        start=(hc == 0), stop=(hc == HC - 1),
                    )
                nc.vector.tensor_scalar_add(
                    dst[hp][:, sl], pk[:], b_sb[:, hp : hp + 1]
                )
                return 1750

            def proj_v(st):
                sb, st4 = st // 4, st % 4
                pv = psum.tile([128, COLS], F32, name="pv", tag="bp")
                for hc in range(HC):
                    nc.tensor.matmul(
                        pv[:], xts[sb][:, hc, st4 * 128 : st4 * 128 + 128],
                        wv_sb[:, hc, :],
                        start=(hc == 0), stop=False,
                    )
                nc.tensor.matmul(
                    pv[:], ones1[:], bv_sb[:], start=False, stop=True
                )
                nc.vector.tensor_copy(
                    Vp[:, st, :, 0:64],
                    pv.rearrange("p (h d) -> p h d", h=LHEADS),
                )
                return 950

            pe_q = deque()

            def drain(budget_ns):
                spent = 0
                while pe_q and spent < budget_ns:
                    fn, ns = pe_q.popleft()
                    fn()
                    spent += ns

            def fill_group(ib, hp, jts):
                isl = slice(ib * SBLK, (ib + 1) * SBLK)
                ng = len(jts)
                tag = "scA" if ng == 2 else "scB"
                sc = psum.tile([128, 2 * ng, SBLK], F32, name=tag, tag=tag)
                for jj in range(ng):
                    for hl in range(2):
                        rows = slice(hl * 64, hl * 64 + 64)
                        nc.tensor.matmul(
                            sc[:, hl * ng + jj, :],
                            KT[hp][rows, jts[jj] * 128 : jts[jj] * 128 + 128],
                            QT[hp][rows, isl],
                            start=True, stop=True,
                        )
                return sc

            gctr = [0]

            def act_group(ib, hp, jts, sc, mt):
                ng = len(jts)
                ptag = "pbig" if ng == 2 else "psml"
                pbufs = pbig_bufs if ng == 2 else psml_bufs
                pg = ppool.tile(
                    [128, 2, ng, SBLK], PDT, name=ptag, tag=ptag, bufs=pbufs
                )
                nc.scalar.activation(
                    pg[:], sc[:],
                    mybir.ActivationFunctionType.Exp, scale=0.125,
                )
                msl = slice(jts[0], jts[0] + ng)
                # every gmask-th group's mask multiply runs on the (otherwise
                # idle) gpsimd engine to keep DVE off the critical path
                gctr[0] += 1
                if gmask == -1:
                    eng = nc.gpsimd if ng == 1 else nc.vector
                else:
                    eng = (
                        nc.gpsimd
                        if gmask and gctr[0] % gmask == 0
                        else nc.vector
                    )
                for hl in range(2):
                    eng.tensor_tensor(
                        pg[:, hl, :, :], pg[:, hl, :, :], mt[:, msl, :],
                        op=mybir.AluOpType.mult,
                    )
                return pg

            def b_unit(hp, jt, pg, jj, po2):
                def emit():
                    for hl in range(2):
                        h = hp * 2 + hl
                        nc.tensor.matmul(
                            po2[hl][:],
                            Vp[:, jt, h, 0:65],
                            pg[:, hl, jj, :],
                            start=(jt == 0), stop=(jt == JT - 1),
                            skip_group_check=True,
                        )
                return emit

            def epi_unit(ib, hp, po2, outt):
                def emit():
                    for hl in range(2):
                        h = hp * 2 + hl
                        po = po2[hl]
                        u = upool.tile([65, SBLK], F32, name="u")
                        nc.vector.tensor_copy(u[:], po[:])
                        pt = psum.tile([128, 4, 65], F32, name="pt", tag="bp")
                        for c in range(4):
                            nc.tensor.transpose(
                                pt[:, c, :],
                                u[:, c * 128 : (c + 1) * 128],
                                ident[0:65, 0:65],
                            )
                        rec = rpool.tile([128, 4], F32, name="rec")
                        nc.vector.reciprocal(rec[:], pt[:, :, 64])
                        for c in range(4):
                            nc.vector.tensor_scalar_mul(
                                outt[:, c, h * 64 : h * 64 + 64],
                                pt[:, c, 0:64],
                                rec[:, c : c + 1],
                            )
                    if hp == 1:
                        isl = slice(ib * SBLK, (ib + 1) * SBLK)
                        nc.sync.dma_start(
                            out[isl, :].rearrange("(c p) n -> p c n", p=128),
                            outt[:],
                        )
                return emit

            for hc in range(HC):
                nc.sync.dma_start(
                    wk_sb[:, hc, :], wk[hc * 128 : (hc + 1) * 128, :]
                )
                nc.sync.dma_start(
                    xts[0][:, hc, :], xT[hc * 128 : (hc + 1) * 128, 0:SBLK]
                )
            load_consts()
            load_w(wq_sb, wq)
            for sb in range(1, NSB):
                load_xt(sb)
            load_w(wv_sb, wv)

            mts = {}

            def load_mask(ib):
                isl = slice(ib * SBLK, (ib + 1) * SBLK)
                mts[ib] = mpool.tile([128, JT, SBLK], PDT, name="mt")
                nc.sync.dma_start(
                    mts[ib][:],
                    maskp[:, isl].rearrange("(t p) i -> p t i", p=128),
                )

            load_mask(0)
            load_mask(1)

            for hp in range(2):
                proj_kq("k", 0, hp)
            for hp in range(2):
                proj_kq("q", 0, hp)

            for sb in range(1, NSB):
                for hp in range(2):
                    pe_q.append(
                        (lambda sb=sb, hp=hp: proj_kq("k", sb, hp), 1750)
                    )
            for st in range(JT):
                pe_q.append((lambda st=st: proj_v(st), 950))
            for hp in range(2):
                pe_q.append((lambda hp=hp: proj_kq("q", 1, hp), 1750))

            for ib in range(NSB):
                if ib + 1 < NSB:
                    load_mask(ib + 1)
                outt = opool.tile([128, 4, COLS], F32, name="outt")
                for hp in range(2):
                    po2 = [
                        psum.tile([65, SBLK], F32, name=f"po{hl}", tag="bp")
                        for hl in range(2)
                    ]
                    for jts in GROUPS:
                        sc = fill_group(ib, hp, jts)
                        pg = act_group(ib, hp, jts, sc, mts[ib])
                        for jj, jt_ in enumerate(jts):
                            pe_q.append(
                                (b_unit(hp, jt_, pg, jj, po2), 430)
                            )
                        act_ns = 2000 if len(jts) == 2 else 1150
                        fill_ns = 860 if len(jts) == 2 else 430
                        drain(act_ns - fill_ns)
                    pe_q.append((epi_unit(ib, hp, po2, outt), 1400))
                if ib + 2 < NSB:
                    for hp in range(2):
                        pe_q.append(
                            (lambda hp=hp, sb=ib + 2: proj_kq("q", sb, hp), 1750)
                        )
            drain(10**9)

    nc.compile()
    return nc


_NC_CACHE = []


def get_nc():
    if not _NC_CACHE:
        _NC_CACHE.append(build_program(fp16=USE_FP16))
    return _NC_CACHE[0]


def make_in_maps(x, attn_mask, Wq, bq, Wk, bk, Wv, bv, fp16=True):
    wdt = np.float16
    pdt = np.float16
    x = np.asarray(x, dtype=np.float32)
    attn_mask = np.asarray(attn_mask)
    Wq, Wk, Wv = (np.asarray(w, dtype=np.float32) for w in (Wq, Wk, Wv))
    bq, bk, bv = (np.asarray(b_, dtype=np.float32) for b_ in (bq, bk, bv))

    in_maps = []
    for core in range(NCORES):
        b = core // CORES_PER_B
        hg = core % CORES_PER_B
        cs = slice(hg * COLS, (hg + 1) * COLS)
        mp = (1 - attn_mask[b].T).astype(pdt)
        in_maps.append(
            {
                "xT": np.ascontiguousarray(x[b].T.astype(wdt)),
                "maskp": np.ascontiguousarray(mp),
                "wq": np.ascontiguousarray(Wq[:, cs].astype(wdt)),
                "wk": np.ascontiguousarray(Wk[:, cs].astype(wdt)),
                "wv": np.ascontiguousarray(Wv[:, cs].astype(wdt)),
                "bq": np.ascontiguousarray(bq[cs, None]),
                "bk": np.ascontiguousarray(bk[cs, None]),
                "bv": np.ascontiguousarray(bv[None, cs].astype(wdt)),
                "ones_d": np.ones((1, 128), wdt),
            }
        )
    return in_maps


def assemble(results):
    out = np.empty((B, S, H), np.float32)
    for core in range(NCORES):
        b = core // CORES_PER_B
        hg = core % CORES_PER_B
        out[b, :, hg * COLS : (hg + 1) * COLS] = results[core]["out"]
    return out


USE_FP16 = True


def kernel(x, attn_mask, Wq, bq, Wk, bk, Wv, bv):
    nc = get_nc()
    in_maps = make_in_maps(x, attn_mask, Wq, bq, Wk, bk, Wv, bv, fp16=USE_FP16)
    res = run_bass_kernel_spmd(nc, in_maps, list(range(NCORES)))
    return assemble(res.results)

